# revision 7
# baseline (speedup 1.0000x reference)
"""MoE (top-1 routing, capacity-dropped) forward on 8 Trainium2 NeuronCores.

bf16 variant of the expert-parallel kernel: weights + activations are cast to
bfloat16 on the host (PE runs bf16 at the same 1-elem/cell/cycle rate as
float32r, so matmul cycle count is unchanged) which
  - halves the weight/activation HBM traffic (33.5 MB -> 16.8 MB per core),
  - enables Fast Weight Load (FWL is disabled for FP32 weights),
  - halves SBUF pressure (h1 goes 16 MB -> 8 MB),
  - draws less PE/SBUF/DMA power (fp32r full-rate is the worst-case power
    mode; sustained high draw downclocks PE 2.4 -> ~2.0 GHz via P0).
Accumulation stays fp32 in PSUM; biases and the final output stay fp32.

Schedule (cost-model timeline 227.3 us vs the 218.5 us PE floor of
1024 matmuls x 512 cycles at the warm 2.4 GHz clock): 16 warm-up matmuls
burn the PE's cold-clock (HAM) window during the initial DMA wait; x and
the first six W1 slabs are issued on one queue in deadline order so the
shared DMA stream never starves the matmul front; mm1 runs k-outer over
2-m-tile groups (4 PSUM banks per group, 8-bank ping-pong) so the first
pass consumes x chunks no faster than they arrive; the final output block
retires as two 256-col groups with the last store on a separate DMA ring.

Strategy (expert-parallel, per the sharding hint):
  - Host computes top-1 gating + capacity dropping and dispatches token rows
    to their expert (tokens sharded along E, one expert per core; W1/b1/W2/b2
    sharded along E; gate Wg applied once on the full token set).
  - Each core: yT = W2[e].T @ relu(W1[e].T @ xT + b1[e]) + b2[e] on its
    C=1024 dispatched tokens in [feature, token] layout.
  - Host combine: scatter expert outputs back scaled by the gate value.

Hardcoded shapes: x [4, 2048, 1024], Wg [1024, 8], W1 [8, 1024, 4096],
b1 [8, 4096], W2 [8, 4096, 1024], b2 [8, 1024].
"""

import os
import numpy as np
from contextlib import ExitStack

try:
    from antenv.axon_hooks import get_axon_ntff_profile_hook  # noqa: F401
except Exception:
    os.environ.setdefault("BASS_NEVER_TRACE", "1")

import ml_dtypes
from concourse import bacc, mybir, tile
from concourse.bass_utils import run_bass_kernel_spmd

B, S, H, F, E = 4, 2048, 1024, 4096, 8
T = B * S                  # 8192 tokens
C = -(-T // E)             # 1024 capacity per expert
P = 128                    # SBUF partitions
NB = 512                   # matmul moving free-dim block (one PSUM bank, fp32)
NCORES = 8

_DT = mybir.dt.bfloat16
_NPDT = ml_dtypes.bfloat16


def _build_expert_ffn(repeat: int = 1):
    """Per-core program: yT = W2.T @ relu(W1.T @ xT + b1) + b2, all [feat, tok].

    repeat>1 wraps the whole body in a hardware For loop — used only by the
    timing harness to measure steady-state per-iteration HW time via the
    wall-clock slope between repeat counts."""
    nc = bacc.Bacc("TRN2", target_bir_lowering=False, debug=False,
                   num_devices=NCORES)
    KC = 8  # k-tiles per streamed W2 chunk (matches the W1 slab size)
    # Weights arrive pre-tiled from the host (see kernel()) so every slab DMA
    # is a fully contiguous read per partition:
    # W1p[m, p, k, f] = W1[k*P+p, m*P+f] and
    # W2p[mh, kc, p, k, f] = W2[kc*KC*P + k*P + p, mh*P + f].
    NBLK = C // NB
    # x arrives host-pretiled as k-chunks of 256 KB, each holding BOTH
    # n-blocks' k-tile (2 KB/partition contiguous): the first matmul waits on
    # one chunk, and at 8 chunks the HWDGE descriptor generation (632 ns/DMA)
    # stays faster than each transfer (728 ns), so delivery is
    # transfer-paced — 16 smaller chunks were descriptor-gen-bound.
    # xT[k, p, n, c] = x_orig[k*P+p, n*NB+c].
    xT = nc.declare_dram_parameter(
        "xT", [H // P, P, NBLK, NB], _DT, isOutput=False)
    W1 = nc.declare_dram_parameter("W1", [F // P, P, H // P, P], _DT, isOutput=False)
    W2 = nc.declare_dram_parameter(
        "W2", [H // P, F // (KC * P), P, KC, P], _DT, isOutput=False)
    b1 = nc.declare_dram_parameter("b1", [P, F // P], mybir.dt.float32,
                                   isOutput=False)  # host-pretiled [p, m]
    b2 = nc.declare_dram_parameter("b2", [P, H // P], mybir.dt.float32,
                                   isOutput=False)
    out = nc.declare_dram_parameter("out", [H, C], mybir.dt.float32, isOutput=True)

    KH = H // P    # 8  k-tiles over H (mm1 contraction)
    KF = F // P    # 32 k-tiles over F (mm2 contraction)
    MF = F // P    # 32 m-tiles over F (mm1 output partitions)
    MH = H // P    # 8  m-tiles over H (mm2 output partitions)

    with tile.TileContext(nc) as tc, ExitStack() as ctx:
        xpool = ctx.enter_context(tc.tile_pool(name="xpool", bufs=1))
        h1pool = ctx.enter_context(tc.tile_pool(name="h1pool", bufs=1))
        cpool = ctx.enter_context(tc.tile_pool(name="cpool", bufs=1))
        wpool = ctx.enter_context(tc.tile_pool(name="wpool", bufs=6))
        ypool = ctx.enter_context(tc.tile_pool(name="ypool", bufs=2))
        # all 8 PSUM banks in one ring: mm1 uses 4 banks per 2-m-tile group,
        # so consecutive groups ping-pong bank quads and a bank's reuse
        # distance (~2 groups of matmuls) far exceeds the activation drain
        psum = ctx.enter_context(tc.tile_pool(name="psum", bufs=8, space="PSUM"))

        # HAM warm-up: the PE clock-gate passes 4/8 pulses (1.2 GHz) until
        # the PE has been busy ~3.4 us. The first real matmul can't start
        # until the first x chunk + W1 slab DMAs land (~3 us), so burn that
        # idle window on throwaway matmuls over a memset tile — the real
        # matmul stream then starts at the warm 2.4 GHz clock. Outside the
        # repeat loop: once per program, like the harness's single shot.
        warm_sb = cpool.tile([P, P], _DT, tag="warm_sb", name="warm_sb")
        nc.vector.memset(warm_sb[:], 0.0)
        warm_ps = psum.tile([P, NB], mybir.dt.float32, tag="ps", name="warm_ps")
        for _ in range(16):
            nc.tensor.matmul(out=warm_ps[:, :P], lhsT=warm_sb[:], rhs=warm_sb[:],
                             start=True, stop=True)

        loop_ctx = tc.For_i(0, repeat, 1) if repeat > 1 else None
        if loop_ctx is not None:
            loop_ctx.__enter__()

        # resident activations + the first four W1 slabs, all on ONE queue in
        # deadline order: the DMA engines drain every queue into one shared-
        # bandwidth stream, so cross-queue round-robin would insert a weight
        # slab between consecutive x chunks and halve x delivery exactly
        # while mm1's first pass races through them. Single-queue issue makes
        # arrival order = deadline order (slab m=2g+j is needed when group g
        # starts; x chunk k is needed 1.7 us per k into group 0).
        x_sb = [xpool.tile([P, NBLK, NB], _DT, tag=f"x_{k}", name=f"x_sb{k}")
                for k in range(KH)]
        early_w1 = {}
        _order = [("s", 0), ("x", 0), ("s", 1), ("x", 1), ("x", 2), ("x", 3),
                  ("s", 2), ("x", 4), ("x", 5), ("x", 6), ("s", 3), ("x", 7),
                  ("s", 4), ("s", 5)]
        for kind, i in _order:
            if kind == "x":
                nc.sync.dma_start(out=x_sb[i][:], in_=xT[i])
            else:
                w1s = wpool.tile([P, KH, P], _DT, tag="wslab", name="w1s")
                nc.sync.dma_start(out=w1s[:], in_=W1[i])
                early_w1[i] = w1s

        # biases arrive host-pretiled: b1[p, m] = b1_orig[m*P + p]
        b1_sb = cpool.tile([P, MF], mybir.dt.float32, name="b1_sb")
        nc.gpsimd.dma_start(out=b1_sb[:], in_=b1[:])
        b2_sb = cpool.tile([P, MH], mybir.dt.float32, name="b2_sb")
        nc.gpsimd.dma_start(out=b2_sb[:], in_=b2[:])

        h1_sb = h1pool.tile([P, KF, C], _DT, name="h1_sb")

        # mm1: h1[m*P+p, c] = relu(b1[m*P+p] + sum_h W1[h, m*P+p] * xT[h, c])
        # k-outer over GROUPS of 2 m-tiles (4 PSUM banks accumulating
        # concurrently): each x chunk k is consumed at 1.7 us of matmul per
        # 0.73 us of chunk arrival, so the first pass over x no longer
        # outruns the shared DMA stream that also carries the W1 slabs
        MG = 2
        for g in range(MF // MG):
            w1g = []
            for j in range(MG):
                m = g * MG + j
                if m in early_w1:
                    w1g.append(early_w1.pop(m))
                    continue
                w1s = wpool.tile([P, KH, P], _DT, tag="wslab", name="w1s")
                nc.scalar.dma_start(out=w1s[:], in_=W1[m])
                w1g.append(w1s)
            ps = [[psum.tile([P, NB], mybir.dt.float32, tag="ps", name="ps")
                   for _ in range(NBLK)] for _ in range(MG)]
            for k in range(KH):
                for j in range(MG):
                    for n in range(NBLK):
                        nc.tensor.matmul(
                            out=ps[j][n][:],
                            lhsT=w1g[j][:, k, :],
                            rhs=x_sb[k][:, n, :],
                            start=(k == 0),
                            stop=(k == KH - 1),
                        )
            for j in range(MG):
                m = g * MG + j
                for n in range(NBLK):
                    nc.scalar.activation(
                        out=h1_sb[:, m, n * NB:(n + 1) * NB],
                        in_=ps[j][n][:],
                        func=mybir.ActivationFunctionType.Relu,
                        bias=b1_sb[:, m:m + 1],
                    )

        # mm2: y[mh*P+p, c] = b2[mh*P+p] + sum_f W2[f, mh*P+p] * h1[f, c]
        for mh in range(MH):
            w2chunks = []
            for kc in range(KF // KC):
                w2s = wpool.tile([P, KC, P], _DT, tag="wslab", name="w2s")
                nc.scalar.dma_start(out=w2s[:], in_=W2[mh, kc])
                w2chunks.append(w2s)
            if mh < MH - 1:
                # k-outer (as in mm1): halves the Ldweights count
                ps2 = [psum.tile([P, NB], mybir.dt.float32, tag="ps", name="ps2")
                       for _ in range(NBLK)]
                for k in range(KF):
                    for n in range(NBLK):
                        nc.tensor.matmul(
                            out=ps2[n][:],
                            lhsT=w2chunks[k // KC][:, k % KC, :],
                            rhs=h1_sb[:, k, n * NB:(n + 1) * NB],
                            start=(k == 0),
                            stop=(k == KF - 1),
                        )
                for n in range(NBLK):
                    y_sb = ypool.tile([P, NB], mybir.dt.float32, tag="y",
                                      name="y_sb")
                    nc.vector.tensor_add(
                        out=y_sb[:],
                        in0=ps2[n][:],
                        in1=b2_sb[:, mh:mh + 1].to_broadcast([P, NB]),
                    )
                    nc.gpsimd.dma_start(
                        out=out[mh * P:(mh + 1) * P, n * NB:(n + 1) * NB],
                        in_=y_sb[:],
                    )
                continue
            # last mh stays n-outer so its n-blocks retire ~7 us apart, and
            # the very last block runs as two 256-col accumulation groups
            # (same total PE cycles) so its first store overlaps the second
            # group's matmuls — halves the exposed drain tail
            for n in range(NBLK):
                last = (n == NBLK - 1)
                SB = NB // 2 if last else NB
                for s in range(NB // SB):
                    c0 = n * NB + s * SB
                    ps2 = psum.tile([P, NB], mybir.dt.float32, tag="ps", name="ps2")
                    for k in range(KF):
                        nc.tensor.matmul(
                            out=ps2[:, :SB],
                            lhsT=w2chunks[k // KC][:, k % KC, :],
                            rhs=h1_sb[:, k, c0:c0 + SB],
                            start=(k == 0),
                            stop=(k == KF - 1),
                        )
                    y_sb = ypool.tile([P, NB], mybir.dt.float32, tag="y", name="y_sb")
                    nc.vector.tensor_add(
                        out=y_sb[:, :SB],
                        in0=ps2[:, :SB],
                        in1=b2_sb[:, mh:mh + 1].to_broadcast([P, SB]),
                    )
                    # final store on the idle SP ring so its descriptor gen +
                    # completion overlap the previous store's on the Pool ring
                    q = nc.sync if (last and s == NB // SB - 1) else nc.gpsimd
                    q.dma_start(
                        out=out[mh * P:(mh + 1) * P, c0:c0 + SB], in_=y_sb[:, :SB]
                    )
        if loop_ctx is not None:
            loop_ctx.__exit__(None, None, None)
    nc.compile()
    return nc


_NC_CACHE = None


def _get_nc():
    global _NC_CACHE
    if _NC_CACHE is None:
        _NC_CACHE = _build_expert_ffn()
    return _NC_CACHE


def _route(tokens: np.ndarray, Wg: np.ndarray):
    """Top-1 gating with capacity C, matching deepspeed top1gating semantics.

    Runs on jax-CPU mirroring the reference ops 1:1 so the discrete routing
    decisions (argmax expert, cumsum slot order, capacity drops) are
    bit-identical to the jax reference — a numpy BLAS logits matmul could
    flip the argmax for tokens whose top-2 logits are ~1e-6 apart.

    Returns per-expert kept token ids (slot order) and per-token gate values
    (0 for dropped tokens)."""
    import jax
    import jax.numpy as jnp

    cpu = jax.devices("cpu")[0]
    with jax.default_device(cpu):
        tok = jnp.asarray(tokens)
        logits = tok @ jnp.asarray(Wg)                       # [T, E]
        gates = jax.nn.softmax(logits, axis=-1)
        idx = jnp.argmax(gates, axis=1)
        mask1 = jax.nn.one_hot(idx, E, dtype=gates.dtype)
        locations1 = jnp.cumsum(mask1, axis=0) - 1.0
        mask1 = mask1 * (locations1 < C).astype(gates.dtype)
        gates1 = jnp.sum(gates * mask1, axis=1)              # 0 if dropped

        mask_np = np.asarray(mask1)
        gate_val = np.asarray(gates1, dtype=np.float32)

    tok_ids = [np.nonzero(mask_np[:, e] > 0)[0] for e in range(E)]
    return tok_ids, gate_val


def _make_in_maps(x, W1, b1, W2, b2, tok_ids):
    """Shard tokens along the expert axis and pre-tile weights to the
    kernel's SBUF slab layout (contiguous per-partition reads), in bf16."""
    KC = 8
    tokens = x.reshape(T, H)
    in_maps = []
    NBLK = C // NB
    for e in range(E):
        xT_e = np.zeros((H, C), dtype=_NPDT)
        ids = tok_ids[e]
        xT_e[:, :len(ids)] = tokens[ids].astype(_NPDT).T
        # kernel SBUF chunk layout: xT[k, p, n, c] = xT_e[k*P+p, n*NB+c]
        xT_t = np.ascontiguousarray(xT_e.reshape(H // P, P, NBLK, NB))
        W1p = np.ascontiguousarray(
            W1[e].reshape(H // P, P, F // P, P).transpose(2, 1, 0, 3)
        ).astype(_NPDT)
        W2p = np.ascontiguousarray(
            W2[e].reshape(F // (KC * P), KC, P, H // P, P).transpose(3, 0, 2, 1, 4)
        ).astype(_NPDT)
        in_maps.append({
            "xT": xT_t,
            "W1": W1p,
            "W2": W2p,
            "b1": np.ascontiguousarray(b1[e].reshape(F // P, P).T),
            "b2": np.ascontiguousarray(b2[e].reshape(H // P, P).T),
        })
    return in_maps


def kernel(x, Wg, W1, b1, W2, b2):
    x = np.asarray(x, dtype=np.float32)
    Wg = np.asarray(Wg, dtype=np.float32)
    W1 = np.asarray(W1, dtype=np.float32)
    b1 = np.asarray(b1, dtype=np.float32)
    W2 = np.asarray(W2, dtype=np.float32)
    b2 = np.asarray(b2, dtype=np.float32)

    tokens = x.reshape(T, H)
    tok_ids, gate_val = _route(tokens, Wg)
    in_maps = _make_in_maps(x, W1, b1, W2, b2, tok_ids)

    nc = _get_nc()
    res = run_bass_kernel_spmd(nc, in_maps, list(range(NCORES)))

    # combine: scatter expert outputs back, scaled by the gate value
    out = np.zeros((T, H), dtype=np.float32)
    for e in range(E):
        ids = tok_ids[e]
        yT = res.results[e]["out"]                   # [H, C]
        out[ids] = yT[:, :len(ids)].T * gate_val[ids, None]
    return out.reshape(B, S, H)
